# revision 3
# baseline (speedup 1.0000x reference)
"""CreateTangentImages kernel v8: dma_gather-based bilinear resample.

Contract: kernel(x, sample_map) -> [B, C, N, gd, gd] f32, matching

    bilinear resample of equirect x [2,3,2048,4096] at sample_map
    [80,256,256,2] (x,y) pixel coords; x wraps horizontally, y clamps.

Strategy:
  - Host: build a record image: one 256B record per (y, x_block) holding a
    2-row x 9-col x 6-channel f16 neighborhood (cols 8*xb .. 8*xb+8, with
    horizontal wrap and vertical clamp baked in). Any bilinear sample with
    x0 in [8*xb, 8*xb+7] and y0 == y needs exactly one record. Per point,
    precompute the record id and an 18-entry zero-padded weight vector
    (2 rows x 9 cols, 4 nonzero bilinear products at the right column).
  - The 80 faces are sharded over 8 cores (10 each); the record image is
    replicated (read-only gather source). Because the custom dma_gather
    instruction takes int16 indices, records are grouped into 32 windows
    of 32768; points are bucketed by window on the host and the output is
    un-permuted on the host afterwards.
  - Device (per core): for each batch of <=8192 points, stream in indices
    and weights, issue one dma_gather (one 256B descriptor per point: the
    fast vectorized Q7 SWDGE path instead of the per-element dynamic DMA
    path), multiply the gathered records by the weights on DVE, reduce
    over the 18 neighborhood slots, and stream out [point, 6ch] f32.
"""

import os
import numpy as np

import concourse.tile as tile
from concourse import bacc, mybir, bass_utils
from concourse.bass_interp import get_hw_module

F32 = mybir.dt.float32
F16 = mybir.dt.float16
I16 = mybir.dt.int16
AX = mybir.AxisListType
OP = mybir.AluOpType

H, W = 2048, 4096
NF, GD = 80, 256
NCORES = 8
FPC = NF // NCORES           # faces per core
PPC = FPC * GD * GD          # points per core

BCOLS = 8                    # x-block width in pixels
XB = W // BCOLS              # 512 record blocks per image row
NREC = H * XB                # 1,048,576 records
RECE = 128                   # f16 elems per record (= 256 bytes)
PAY = 108                    # payload elems: 2 rows x 9 cols x 6 ch
WIN = 32768                  # records per gather window (int16 index range)
NWIN = NREC // WIN           # 32
NI_MAX = 8192                # points per dma_gather call

_cache = {}
_prep_cache = {}
last_exec_time_ns = None


def _build_program(caps):
    S = sum(caps)
    nc = bacc.Bacc("TRN2", target_bir_lowering=False, debug=False,
                   enable_asserts=False, dynamic_dma_scratch_size=32768)
    img = nc.dram_tensor("img", [NREC, RECE], F16, kind="ExternalInput")
    idxd = nc.dram_tensor("idxd", [128, S // 16], I16, kind="ExternalInput")
    wd = nc.dram_tensor("wd", [128, S // 128, 18], F16, kind="ExternalInput")
    outd = nc.dram_tensor("outd", [128, S // 128, 6], F32,
                          kind="ExternalOutput")

    calls = _call_list(caps)

    with tile.TileContext(nc) as tc:
        with (
            tc.tile_pool(name="io", bufs=3) as iop,
            tc.tile_pool(name="g", bufs=3) as gp,
            tc.tile_pool(name="o", bufs=3) as op_,
        ):
            for (w, c0, ni) in calls:
                J = ni // 128
                it = iop.tile([128, NI_MAX // 16], I16, tag="idx")
                nc.sync.dma_start(out=it[:, :ni // 16],
                                  in_=idxd[:, c0 * 8:c0 * 8 + ni // 16])
                wt = iop.tile([128, NI_MAX // 128 * 18], F16, tag="w")
                wtv = wt[:].rearrange("p (j s) -> p j s", s=18)[:, :J, :]
                nc.sync.dma_start(out=wtv, in_=wd[:, c0:c0 + J, :])

                g = gp.tile([128, NI_MAX // 128 * RECE], F16, tag="g")
                gv = g[:].rearrange("p (j e) -> p j e", e=RECE)[:, :J, :]
                nc.gpsimd.dma_gather(
                    out_ap=gv,
                    in_ap=img[w * WIN:(w + 1) * WIN],
                    idxs_ap=it[:, :ni // 16],
                    num_idxs=ni,
                    num_idxs_reg=ni,
                    elem_size=RECE,
                    # >64 descriptors per engine overflows the SDMA packet
                    # limit when coalesced; one packet per descriptor.
                    single_packet=False,
                )

                mv = gv[:, :, 0:PAY].rearrange("p j (s c) -> p j s c", c=6)
                wb = wtv.unsqueeze(3).to_broadcast([128, J, 18, 6])
                nc.vector.tensor_tensor(out=mv, in0=mv, in1=wb, op=OP.mult)

                ov = op_.tile([128, NI_MAX // 128 * 6], F32, tag="o")
                o6v = ov[:].rearrange("p (j c) -> p j c", c=6)[:, :J, :]
                nc.vector.tensor_reduce(out=o6v, in_=mv.transpose([0, 1, 3, 2]),
                                        axis=AX.X, op=OP.add)
                nc.sync.dma_start(out=outd[:, c0:c0 + J, :], in_=o6v)

    nc.compile()
    nc.m = get_hw_module(nc.m)
    return nc


def _call_list(caps):
    calls = []
    c0 = 0
    for w, cap in enumerate(caps):
        off = 0
        while off < cap:
            ni = min(NI_MAX, cap - off)
            calls.append((w, c0, ni))
            c0 += ni // 128
            off += ni
    return calls


def _get_program(caps):
    key = tuple(caps)
    if key not in _cache:
        _cache[key] = _build_program(key)
    return _cache[key]


def _build_img(x):
    """[NREC, RECE] f16 record image; record (y, xb) = rows {y, y+1 clamped}
    x cols {8xb..8xb+8 wrapped} x 6 ch, layout [row][col][ch] + pad."""
    a16 = np.asarray(x, np.float32).reshape(6, H, W).astype(np.float16)
    img6 = np.ascontiguousarray(a16.transpose(1, 2, 0))       # [H, W, 6]
    r1 = img6[np.minimum(np.arange(H) + 1, H - 1)]            # [H, W, 6]
    out = np.zeros((H, XB, RECE), np.float16)
    base = np.arange(XB) * BCOLS
    for d in range(9):
        cols = (base + d) % W
        out[:, :, d * 6:d * 6 + 6] = img6[:, cols]
        out[:, :, 54 + d * 6:54 + d * 6 + 6] = r1[:, cols]
    return out.reshape(NREC, RECE)


def _point_tables(sample_map):
    sm = np.asarray(sample_map, np.float32)
    sx = sm[..., 0].reshape(-1)
    sy = sm[..., 1].reshape(-1)
    x0 = np.floor(sx)
    y0 = np.floor(sy)
    wx = sx - x0
    wy = sy - y0
    x0i = (x0.astype(np.int64)) % W
    y0i = np.clip(y0.astype(np.int64), 0, H - 1)
    xb = x0i >> 3
    dx = x0i & 7
    rec = y0i * XB + xb
    win = (rec >> 15).astype(np.int32)
    widx = (rec & (WIN - 1)).astype(np.int16)
    P = sx.shape[0]
    w18 = np.zeros((P, 18), np.float16)
    ar = np.arange(P)
    omx = 1.0 - wx
    omy = 1.0 - wy
    w18[ar, dx] = omx * omy
    w18[ar, dx + 1] = wx * omy
    w18[ar, 9 + dx] = omx * wy
    w18[ar, 10 + dx] = wx * wy
    return win, widx, w18


def _prepare(x, sample_map):
    """Host precompute: record image, per-core bucketed index/weight streams,
    per-window capacities, and the stream-slot permutation for decode."""
    img = _build_img(x)
    win, widx, w18 = _point_tables(sample_map)
    win_c = win.reshape(NCORES, PPC)
    widx_c = widx.reshape(NCORES, PPC)
    w18_c = w18.reshape(NCORES, PPC, 18)

    counts = np.stack([np.bincount(win_c[c], minlength=NWIN)
                       for c in range(NCORES)])                 # [8, NWIN]
    caps = tuple(int(np.ceil(max(1, int(counts[:, w].max())) / 128) * 128)
                 for w in range(NWIN))
    S = sum(caps)
    sec_off = np.concatenate([[0], np.cumsum(caps)])[:-1]       # [NWIN]
    calls = _call_list(caps)

    in_maps = []
    slots = []
    for c in range(NCORES):
        order = np.argsort(win_c[c], kind="stable")
        starts = np.concatenate([[0], np.cumsum(counts[c])])[:-1]
        win_sorted = win_c[c][order]
        rank = np.arange(PPC) - starts[win_sorted]
        slot_sorted = sec_off[win_sorted] + rank                # [PPC]

        idx_stream = np.zeros(S, np.int16)
        idx_stream[slot_sorted] = widx_c[c][order]
        w_stream = np.zeros((S, 18), np.float16)
        w_stream[slot_sorted] = w18_c[c][order]

        stream_slots = np.empty(PPC, np.int64)
        stream_slots[order] = slot_sorted
        slots.append(stream_slots)

        blocks = []
        for (w, c0, ni) in calls:
            seg = idx_stream[c0 * 128:c0 * 128 + ni]
            blocks.append(seg.reshape(ni // 16, 16).T)          # [16, ni/16]
        idx16 = np.concatenate(blocks, axis=1)                  # [16, S/16]
        idxd = np.ascontiguousarray(np.tile(idx16, (8, 1)))     # [128, S/16]

        wdv = np.ascontiguousarray(
            w_stream.reshape(S // 128, 128, 18).transpose(1, 0, 2))

        in_maps.append({"img": img, "idxd": idxd, "wd": wdv})

    return caps, S, in_maps, slots


def _prepare_cached(x, sample_map):
    x = np.ascontiguousarray(np.asarray(x, dtype=np.float32))
    sample_map = np.ascontiguousarray(np.asarray(sample_map, dtype=np.float32))
    assert x.shape == (2, 3, H, W), x.shape
    assert sample_map.shape == (NF, GD, GD, 2), sample_map.shape
    hit = _prep_cache.get("k")
    if hit is not None:
        px, psm, prep = hit
        if np.array_equal(px, x) and np.array_equal(psm, sample_map):
            return prep
    prep = _prepare(x, sample_map)
    _prep_cache["k"] = (x, sample_map, prep)
    return prep


def _decode(outd, stream_slots):
    """[128, S/128, 6] f32 device block -> [PPC, 6] in original point order."""
    S = outd.shape[1] * 128
    stream = outd.transpose(1, 0, 2).reshape(S, 6)
    return stream[stream_slots]


def kernel(x, sample_map):
    global last_exec_time_ns
    caps, S, in_maps, slots = _prepare_cached(x, sample_map)
    nc = _get_program(caps)
    trace = bool(int(os.environ.get("TANGENT_TRACE", "0")))
    res = bass_utils.run_bass_kernel_spmd(
        nc, in_maps, core_ids=list(range(NCORES)), trace=trace
    )
    last_exec_time_ns = res.exec_time_ns

    full = np.empty((2, 3, NF, GD, GD), dtype=np.float32)
    for core in range(NCORES):
        pts = _decode(np.asarray(res.results[core]["outd"]), slots[core])
        oc = pts.T.reshape(6, FPC, GD, GD)
        full[:, :, core * FPC:(core + 1) * FPC] = oc.reshape(2, 3, FPC, GD, GD)
    return full


def measure_exec_ns(x, sample_map, n_chain=3, iters=2):
    """Device-resident slope timing: run the NEFF once and n_chain times
    inside single dispatches; the slope is the per-execution device time
    (axon dispatch overhead cancels). Returns ns."""
    import time
    import jax
    from jax.sharding import Mesh, PartitionSpec
    from jax.experimental.shard_map import shard_map
    from concourse import bass2jax

    caps, S, in_maps, slots = _prepare_cached(x, sample_map)
    nc = _get_program(caps)
    bass2jax.install_neuronx_cc_hook()
    partition_name = nc.partition_id_tensor.name if nc.partition_id_tensor else None
    in_names, out_names, out_avals, zero_outs = [], [], [], []
    for alloc in nc.m.functions[0].allocations:
        if not isinstance(alloc, mybir.MemoryLocationSet):
            continue
        name = alloc.memorylocations[0].name
        if alloc.kind == "ExternalInput":
            if name != partition_name:
                in_names.append(name)
        elif alloc.kind == "ExternalOutput":
            out_names.append(name)
            shape = tuple(alloc.tensor_shape)
            dtype = mybir.dt.np(alloc.dtype)
            out_avals.append(jax.core.ShapedArray(shape, dtype))
            zero_outs.append(np.zeros(shape, dtype))
    n_params, n_outs = len(in_names), len(out_avals)
    all_names = in_names + out_names + ([partition_name] if partition_name else [])

    devices = jax.devices()[:NCORES]
    mesh = Mesh(np.asarray(devices), ("core",))

    def _body(*args):
        operands = list(args)
        if partition_name is not None:
            operands.append(bass2jax.partition_id_tensor())
        return tuple(bass2jax._bass_exec_p.bind(
            *operands,
            out_avals=tuple(out_avals),
            in_names=tuple(all_names),
            out_names=tuple(out_names),
            lowering_input_output_aliases=(),
            sim_require_finite=True,
            sim_require_nnan=True,
            nc=nc,
        ))

    f = jax.jit(
        shard_map(_body, mesh=mesh,
                  in_specs=(PartitionSpec("core"),) * (n_params + n_outs),
                  out_specs=(PartitionSpec("core"),) * n_outs, check_rep=False),
        donate_argnums=tuple(range(n_params, n_params + n_outs)),
        keep_unused=True,
    )

    concat_in = [
        np.concatenate([np.asarray(in_maps[c][n]) for c in range(NCORES)], axis=0)
        for n in in_names
    ]
    dev_in = [jax.device_put(a) for a in concat_in]
    for a in dev_in:
        a.block_until_ready()

    def run(k):
        """Queue k async dispatches, block once; min over iters."""
        best = None
        for _ in range(iters):
            zsets = []
            for _ in range(k):
                zo = [jax.device_put(np.concatenate([z] * NCORES, axis=0))
                      for z in zero_outs]
                for a in zo:
                    a.block_until_ready()
                zsets.append(zo)
            t0 = time.time()
            allouts = [f(*dev_in, *zo) for zo in zsets]
            for outs in allouts:
                for o in outs:
                    o.block_until_ready()
            dt = time.time() - t0
            best = dt if best is None else min(best, dt)
        return best

    run(1)  # warmup (includes NEFF compile)
    t1 = run(1)
    tn = run(n_chain)
    return max(0.0, (tn - t1) / (n_chain - 1)) * 1e9


# revision 9
# speedup vs baseline: 2.2222x; 2.2222x over previous
"""CreateTangentImages kernel v9: band-sharded image, minimal I/O.

Contract: kernel(x, sample_map) -> [B, C, N, gd, gd] f32, matching

    bilinear resample of equirect x [2,3,2048,4096] at sample_map
    [80,256,256,2] (x,y) pixel coords; x wraps horizontally, y clamps.

The dominant cost of a dispatch on these axon-tunneled cores is per-core
input staging (~0.4 ms/MB) on top of a fixed overhead, so the kernel
minimizes bytes shipped per core:

  - The equirect image is sharded into 8 horizontal bands of 256 rows
    (not replicated): each core gets a 12.6MB int8 "vertical pairs" band
    imgp[yl*W+x] = concat(img6[y,x,:], img6[y+1,x,:]) for its 256 rows,
    with the vertical clamp baked in. Points are bucketed by the band
    containing floor(y) on the host; sy is shipted band-local so one SPMD
    program serves all cores. Outputs are un-permuted on the host.
  - Coordinates ship as raw f32 (8B/point); the output returns as f16.
  - Device per point tile (128x512): floor/frac + corner weights on DVE,
    one indirect 24B gather per 128-point group (4 corners = 24 contiguous
    int8 under the pairs layout), weighted corner reduce, f16 out.
"""

import os
import numpy as np

import concourse.tile as tile
from concourse import bacc, mybir, bass_utils
from concourse.bass import IndirectOffsetOnAxis
from concourse.bass_interp import get_hw_module

F32 = mybir.dt.float32
F16 = mybir.dt.float16
I8 = mybir.dt.int8
I32 = mybir.dt.int32
AX = mybir.AxisListType
OP = mybir.AluOpType

H, W = 2048, 4096
NF, GD = 80, 256
NCORES = 8
PTOT = NF * GD * GD          # 5,242,880 points total
BH = H // NCORES             # band height: 256 rows per core
Q = 512                      # points per tile column dim
TILE = 128 * Q               # points per tile

_cache = {}
_prep_cache = {}
last_exec_time_ns = None
last_results = None


def _build_program(t_tiles, dq):
    nc = bacc.Bacc("TRN2", target_bir_lowering=False, debug=False,
                   enable_asserts=False)
    imgp = nc.dram_tensor("imgp", [BH * W, 12], I8, kind="ExternalInput")
    smx = nc.dram_tensor("smx", [t_tiles, 128, Q], F32, kind="ExternalInput")
    smy = nc.dram_tensor("smy", [t_tiles, 128, Q], F32, kind="ExternalInput")
    out = nc.dram_tensor("out", [t_tiles, 128, Q * 6], F16,
                         kind="ExternalOutput")

    CHUNK = 64
    n_chunks = Q // CHUNK

    with tile.TileContext(nc) as tc:
        with (
            tc.tile_pool(name="sm", bufs=2) as smp,
            tc.tile_pool(name="idx", bufs=2) as idxp,
            tc.tile_pool(name="gat", bufs=3) as gp,
            tc.tile_pool(name="o", bufs=2) as op,
        ):
            for t in range(t_tiles):
                sx = smp.tile([128, Q], F32, tag="sx")
                nc.sync.dma_start(out=sx[:], in_=smx[t])
                sy = smp.tile([128, Q], F32, tag="sy")
                nc.sync.dma_start(out=sy[:], in_=smy[t])

                # floor via int cast (HW rounds to nearest) + is_gt fixup
                xi = idxp.tile([128, Q], I32, tag="xi")
                nc.vector.tensor_copy(out=xi[:], in_=sx[:])
                xf = idxp.tile([128, Q], F32, tag="xf")
                nc.vector.tensor_copy(out=xf[:], in_=xi[:])
                fx = idxp.tile([128, Q], F32, tag="fx")
                nc.vector.tensor_tensor(out=fx[:], in0=xf[:], in1=sx[:], op=OP.is_gt)
                nc.vector.tensor_tensor(out=xf[:], in0=xf[:], in1=fx[:], op=OP.subtract)

                yi = idxp.tile([128, Q], I32, tag="yi")
                nc.vector.tensor_copy(out=yi[:], in_=sy[:])
                yf = idxp.tile([128, Q], F32, tag="yf")
                nc.vector.tensor_copy(out=yf[:], in_=yi[:])
                fy = idxp.tile([128, Q], F32, tag="fy")
                nc.vector.tensor_tensor(out=fy[:], in0=yf[:], in1=sy[:], op=OP.is_gt)
                nc.vector.tensor_tensor(out=yf[:], in0=yf[:], in1=fy[:], op=OP.subtract)

                wx = idxp.tile([128, Q], F32, tag="wx")
                nc.vector.tensor_tensor(out=wx[:], in0=sx[:], in1=xf[:], op=OP.subtract)
                wy = idxp.tile([128, Q], F32, tag="wy")
                nc.vector.tensor_tensor(out=wy[:], in0=sy[:], in1=yf[:], op=OP.subtract)
                nc.vector.tensor_scalar_min(out=xf[:], in0=xf[:], scalar1=float(W - 2))
                nc.vector.tensor_scalar_min(out=yf[:], in0=yf[:],
                                            scalar1=float(BH - 1))

                idxf = idxp.tile([128, Q], F32, tag="idxf")
                nc.vector.tensor_scalar_mul(out=idxf[:], in0=yf[:], scalar1=float(W))
                nc.vector.tensor_tensor(out=idxf[:], in0=idxf[:], in1=xf[:], op=OP.add)
                idxi = idxp.tile([128, Q], I32, tag="idxi")
                nc.vector.tensor_copy(out=idxi[:], in_=idxf[:])

                omx = idxp.tile([128, Q], F32, tag="omx")
                nc.vector.tensor_scalar(out=omx[:], in0=wx[:], scalar1=-1.0,
                                        scalar2=1.0, op0=OP.mult, op1=OP.add)
                omy = idxp.tile([128, Q], F32, tag="omy")
                nc.vector.tensor_scalar(out=omy[:], in0=wy[:], scalar1=-1.0,
                                        scalar2=1.0, op0=OP.mult, op1=OP.add)

                # corner weights interleaved [w00, w10, w01, w11] per point
                w4 = idxp.tile([128, Q * 4], F16, tag="w4")
                w4v = w4[:].rearrange("p (q f) -> p q f", f=4)
                nc.vector.tensor_tensor(out=w4v[:, :, 0], in0=omx[:], in1=omy[:], op=OP.mult)
                nc.vector.tensor_tensor(out=w4v[:, :, 1], in0=omx[:], in1=wy[:], op=OP.mult)
                nc.vector.tensor_tensor(out=w4v[:, :, 2], in0=wx[:], in1=omy[:], op=OP.mult)
                nc.vector.tensor_tensor(out=w4v[:, :, 3], in0=wx[:], in1=wy[:], op=OP.mult)

                o6 = op.tile([128, Q * 6], F16, tag="o6")

                for c in range(n_chunks):
                    data = gp.tile([128, CHUNK * 24], I8, tag="data")
                    for j in range(CHUNK):
                        qq = c * CHUNK + j
                        nc.gpsimd.indirect_dma_start(
                            out=data[:, j * 24:(j + 1) * 24],
                            out_offset=None,
                            in_=imgp[:],
                            in_offset=IndirectOffsetOnAxis(ap=idxi[:, qq:qq + 1], axis=0),
                        )
                    dataf = gp.tile([128, CHUNK * 24], F16, tag="dataf")
                    nc.vector.tensor_copy(out=dataf[:], in_=data[:])
                    datav = dataf[:].rearrange("p (q s c) -> p q s c", s=4, c=6)
                    w4b = (w4v[:, c * CHUNK:(c + 1) * CHUNK, :]
                           .unsqueeze(3).to_broadcast([128, CHUNK, 4, 6]))
                    nc.vector.tensor_tensor(out=datav, in0=datav, in1=w4b, op=OP.mult)
                    red_in = datav.transpose([0, 1, 3, 2])  # [128, CHUNK, 6, 4]
                    o6v = (o6[:, c * CHUNK * 6:(c + 1) * CHUNK * 6]
                           .rearrange("p (q c) -> p q c", c=6))
                    with nc.allow_low_precision(reason="f16 out; 4-term sum"):
                        nc.vector.tensor_reduce(out=o6v, in_=red_in,
                                                axis=AX.X, op=OP.add)

                nc.vector.tensor_scalar_mul(out=o6[:], in0=o6[:], scalar1=float(dq))
                nc.sync.dma_start(out=out[t], in_=o6[:])

    nc.compile()
    nc.m = get_hw_module(nc.m)
    return nc


def _get_program(t_tiles, dq):
    key = (int(t_tiles), float(dq))
    if key not in _cache:
        _cache[key] = _build_program(t_tiles, dq)
    return _cache[key]


def _build_bands(x):
    """Per-core pairs band [BH*W, 12] int8: rows 256c..256c+255, each record
    = 6ch at (y,x) + 6ch at (min(y+1,H-1),x)."""
    img6 = np.ascontiguousarray(x.reshape(6, H, W).transpose(1, 2, 0))
    down = img6[np.minimum(np.arange(H) + 1, H - 1)]
    imgp = np.concatenate([img6, down], axis=2)      # [H, W, 12] f32
    s = float(np.abs(imgp).max()) or 1.0
    q = np.clip(np.round(imgp * (127.0 / s)), -127, 127).astype(np.int8)
    bands = [np.ascontiguousarray(q[c * BH:(c + 1) * BH].reshape(BH * W, 12))
             for c in range(NCORES)]
    return bands, s / 127.0


def _prepare(x, sample_map):
    bands, dq = _build_bands(x)
    sm = np.asarray(sample_map, np.float32)
    sx = sm[..., 0].reshape(-1)
    sy = sm[..., 1].reshape(-1)
    y0 = np.clip(np.floor(sy).astype(np.int64), 0, H - 1)
    band = np.minimum(y0 // BH, NCORES - 1).astype(np.int32)

    counts = np.bincount(band, minlength=NCORES)
    t_tiles = int(max(1, -(-int(counts.max()) // TILE)))
    S = t_tiles * TILE

    in_maps, sels = [], []
    for c in range(NCORES):
        sel = np.nonzero(band == c)[0]
        n = sel.shape[0]
        sxs = np.zeros(S, np.float32)
        sys_ = np.zeros(S, np.float32)
        sxs[:n] = sx[sel]
        sys_[:n] = sy[sel] - float(c * BH)   # band-local y
        in_maps.append({
            "imgp": bands[c],
            "smx": np.ascontiguousarray(sxs.reshape(t_tiles, 128, Q)),
            "smy": np.ascontiguousarray(sys_.reshape(t_tiles, 128, Q)),
        })
        sels.append(sel)
    return t_tiles, dq, in_maps, sels


def _prepare_cached(x, sample_map):
    x = np.ascontiguousarray(np.asarray(x, dtype=np.float32))
    sample_map = np.ascontiguousarray(np.asarray(sample_map, dtype=np.float32))
    assert x.shape == (2, 3, H, W), x.shape
    assert sample_map.shape == (NF, GD, GD, 2), sample_map.shape
    hit = _prep_cache.get("k")
    if hit is not None:
        px, psm, prep = hit
        if np.array_equal(px, x) and np.array_equal(psm, sample_map):
            return prep
    prep = _prepare(x, sample_map)
    _prep_cache["k"] = (x, sample_map, prep)
    return prep


def kernel(x, sample_map):
    global last_exec_time_ns, last_results
    t_tiles, dq, in_maps, sels = _prepare_cached(x, sample_map)
    nc = _get_program(t_tiles, dq)
    trace = bool(int(os.environ.get("TANGENT_TRACE", "0")))
    res = bass_utils.run_bass_kernel_spmd(
        nc, in_maps, core_ids=list(range(NCORES)), trace=trace
    )
    last_exec_time_ns = res.exec_time_ns
    last_results = res

    flat = np.empty((PTOT, 6), dtype=np.float32)
    for core in range(NCORES):
        o = np.asarray(res.results[core]["out"]).astype(np.float32)
        # out[t, p, q*6+c]; host stream position s = (t, p, q) C-order
        pts = o.reshape(-1, 6)
        flat[sels[core]] = pts[:len(sels[core])]
    full = flat.T.reshape(6, NF, GD, GD).reshape(2, 3, NF, GD, GD)
    return full


def measure_exec_ns(x, sample_map, n_chain=3, iters=2):
    """Device-resident slope timing: run the NEFF once and n_chain times
    inside single dispatches; the slope is the per-execution device time
    (axon dispatch overhead cancels). Returns ns."""
    import time
    import jax
    from jax.sharding import Mesh, PartitionSpec
    from jax.experimental.shard_map import shard_map
    from concourse import bass2jax

    t_tiles, dq, in_maps, sels = _prepare_cached(x, sample_map)
    nc = _get_program(t_tiles, dq)
    bass2jax.install_neuronx_cc_hook()
    partition_name = nc.partition_id_tensor.name if nc.partition_id_tensor else None
    in_names, out_names, out_avals, zero_outs = [], [], [], []
    for alloc in nc.m.functions[0].allocations:
        if not isinstance(alloc, mybir.MemoryLocationSet):
            continue
        name = alloc.memorylocations[0].name
        if alloc.kind == "ExternalInput":
            if name != partition_name:
                in_names.append(name)
        elif alloc.kind == "ExternalOutput":
            out_names.append(name)
            shape = tuple(alloc.tensor_shape)
            dtype = mybir.dt.np(alloc.dtype)
            out_avals.append(jax.core.ShapedArray(shape, dtype))
            zero_outs.append(np.zeros(shape, dtype))
    n_params, n_outs = len(in_names), len(out_avals)
    all_names = in_names + out_names + ([partition_name] if partition_name else [])

    devices = jax.devices()[:NCORES]
    mesh = Mesh(np.asarray(devices), ("core",))

    def _body(*args):
        operands = list(args)
        if partition_name is not None:
            operands.append(bass2jax.partition_id_tensor())
        return tuple(bass2jax._bass_exec_p.bind(
            *operands,
            out_avals=tuple(out_avals),
            in_names=tuple(all_names),
            out_names=tuple(out_names),
            lowering_input_output_aliases=(),
            sim_require_finite=True,
            sim_require_nnan=True,
            nc=nc,
        ))

    f = jax.jit(
        shard_map(_body, mesh=mesh,
                  in_specs=(PartitionSpec("core"),) * (n_params + n_outs),
                  out_specs=(PartitionSpec("core"),) * n_outs, check_rep=False),
        donate_argnums=tuple(range(n_params, n_params + n_outs)),
        keep_unused=True,
    )

    concat_in = [
        np.concatenate([np.asarray(in_maps[c][n]) for c in range(NCORES)], axis=0)
        for n in in_names
    ]
    dev_in = [jax.device_put(a) for a in concat_in]
    for a in dev_in:
        a.block_until_ready()

    def run(k):
        """Queue k async dispatches, block once; min over iters."""
        best = None
        for _ in range(iters):
            zsets = []
            for _ in range(k):
                zo = [jax.device_put(np.concatenate([z] * NCORES, axis=0))
                      for z in zero_outs]
                for a in zo:
                    a.block_until_ready()
                zsets.append(zo)
            t0 = time.time()
            allouts = [f(*dev_in, *zo) for zo in zsets]
            for outs in allouts:
                for o in outs:
                    o.block_until_ready()
            dt = time.time() - t0
            best = dt if best is None else min(best, dt)
        return best

    run(1)  # warmup (includes NEFF compile)
    t1 = run(1)
    tn = run(n_chain)
    return max(0.0, (tn - t1) / (n_chain - 1)) * 1e9
